# revision 4
# baseline (speedup 1.0000x reference)
"""Trainium2 Bass kernel for ConcatHandshaking.

out[b, p, :] = tanh(hidden[b, i_p] @ W1.T + hidden[b, j_p] @ W2.T + fc_b)
for the S*(S+1)/2 upper-triangular pairs (i_p, j_p), i-major order.

Device layout: output features (H=768) on SBUF partitions, pair index on the
free dim.  Per stripe (128 features) the pre-activation for pair (i, j) is
q2[:, j] + p1[:, i]; the work is split by segment length:

- middle segments (i < TI): one DVE tensor_scalar_add per segment (runs in
  2x perf mode: bf16 tensor operands, f32 per-partition scalar), then one
  big ACT tanh per ~4K-column chunk, then ~1MB DMAs.  Segment lengths are
  padded to even so every bf16 AP is 4B-aligned (host discards pad cols);
  odd-start reads use q2s, a one-column-shifted copy of q2.
- tail segments (i >= TI = 129, lengths <= 127): per-op overhead would
  drown DVE (and ACT), so the tensor engine computes them instead:
  psum[f, t] = sum_k STK[k, f] * IND[k, t] where STK stacks [p1T rows;
  p2T rows; fc_b row] (built on-chip via transposed matmuls) and IND is a
  constant 0/1 bf16 matrix streamed from HBM (IND_A: i-indicators,
  IND_B: j-indicators + ones row for the bias).  ACT tanh reads the f32
  PSUM chunks directly and writes bf16 to SBUF.

Everything after PSUM is bf16: halves DMA write traffic (HBM ~358GB/s/core
would otherwise bound at ~140us) and doubles DVE throughput.

Sharding (8 cores): core k handles batch b = k//2 and output-feature rows
[384*(k%2), 384*(k%2)+384) -> 3 stripes of [128 features, P pairs] each.
Per-core DRAM output is (3, 128, PPAD) bf16; host gathers the packed
columns, upcasts to f32 and transposes.
"""

import sys

import numpy as np

for _p in ("/opt/trn_rl_repo",):
    if _p not in sys.path:
        sys.path.insert(0, _p)

B, S, H = 4, 256, 768
P = S * (S + 1) // 2  # 32896
KT = H // 128  # 6 k-tiles
OC = 3  # o-chunks (of 128) per core
# bf16 packed matmul input columns: [ ht (S) | w1t (384) | w2t (384) ]
W1C = S
W2C = S + 128 * OC
IC16 = S + 2 * 128 * OC  # 1024

TI = 129  # tail threshold: segments i >= TI go through the tensor engine
NTAIL = S - TI  # 127 tail i-rows (and 127 tail j-values TI..255)
TAILN = NTAIL * (NTAIL + 1) // 2  # 8128 packed tail columns per stripe
TPCH = 1024  # tail PSUM chunk columns (2 banks)
NCHUNK = 6  # middle tanh/DMA chunks per stripe (~1MB bf16 DMAs)

_NC_CACHE = {}
_LAYOUT_CACHE = {}
LAST = {}


def _layout(nchunk=NCHUNK, lead_split=True):
    """Padded middle layout + chunking + packed tail, and the host gather map.

    Middle segment i (i < TI) holds pairs (i, j) j=i..S-1, length L=S-i,
    padded to even Lp.  Chunk boundaries snap to segment starts.  The tail
    region (packed, TAILN cols) sits after the middle.

    Returns (PPAD, MIDP, chunks, idx): chunks is a list per middle chunk of
    (coff, csz, [(i, src_sel, src_off, Lp, dst0), ...]); idx maps packed
    column -> padded column for the host-side gather.
    """
    key = (nchunk, lead_split)
    if key in _LAYOUT_CACHE:
        return _LAYOUT_CACHE[key]
    segs = []
    pos = 0
    for i in range(TI):
        L = S - i
        Lp = L + (L & 1)
        segs.append((i, pos, L, Lp))
        pos += Lp
    midp = pos  # 24832
    ppad = midp + TAILN  # 32960
    targets = [round(midp * t / nchunk) for t in range(1, nchunk)]
    bounds = [0]
    for t in targets:
        b = min((s[1] for s in segs), key=lambda x: abs(x - t))
        if b > bounds[-1]:
            bounds.append(b)
    bounds.append(midp)
    if lead_split:
        # split the first chunk in half for earlier pipeline start
        half = min(
            (s[1] for s in segs if 0 < s[1] < bounds[1]),
            key=lambda x: abs(x - bounds[1] // 2),
        )
        bounds = [0, half] + bounds[1:]
    chunks = []
    idx = np.empty(P, dtype=np.int64)
    poff = 0
    si = 0
    for c0, c1 in zip(bounds[:-1], bounds[1:]):
        ops = []
        while si < len(segs) and segs[si][1] < c1:
            i, dstart, L, Lp = segs[si]
            idx[poff : poff + L] = dstart + np.arange(L)
            poff += L
            if i % 2 == 0:
                ops.append((i, 0, i, Lp, dstart - c0))
            else:
                ops.append((i, 1, i - 1, Lp, dstart - c0))
            si += 1
        chunks.append((c0, c1 - c0, ops))
    idx[poff:] = midp + np.arange(TAILN)
    _LAYOUT_CACHE[key] = (ppad, midp, chunks, idx)
    return _LAYOUT_CACHE[key]


def _tail_pairs():
    ii, jj = [], []
    for i in range(TI, S):
        for j in range(i, S):
            ii.append(i)
            jj.append(j)
    return np.array(ii), np.array(jj)


def _build_nc(loop_k=None, nchunk=NCHUNK):
    import contextlib

    import concourse.bacc as bacc
    import concourse.bass as bass
    import concourse.mybir as mybir
    import concourse.tile as tile

    f32 = mybir.dt.float32
    bf16 = mybir.dt.bfloat16
    ppad, midp, chunks, _ = _layout(nchunk)
    cmax = max(c[1] for c in chunks)
    ntp = (TAILN + TPCH - 1) // TPCH  # tail psum chunks per stripe

    # Bacc (not raw Bass): its compile() runs generate_event_semaphores,
    # which splits multi-sem waits to satisfy TRN2's 1-wait-per-instruction.
    nc = bacc.Bacc()

    inp16_d = nc.declare_dram_parameter("inp16", [H, IC16], bf16, isOutput=False)
    # f32 side data: col 0 = fcb (rows 0:384), col 1 = zeros
    aux_d = nc.declare_dram_parameter("aux", [H, 2], f32, isOutput=False)
    # constant 0/1 indicator matrices for the tail gather-matmul
    inda_d = nc.declare_dram_parameter("ind_a", [NTAIL, TAILN], bf16, isOutput=False)
    indb_d = nc.declare_dram_parameter("ind_b", [128, TAILN], bf16, isOutput=False)
    # per-stripe fc_b as a row (partition 127 of STK_B)
    brow_d = nc.declare_dram_parameter("brow", [OC, 128], bf16, isOutput=False)
    out_d = nc.declare_dram_parameter("out", [OC, 128, ppad], bf16, isOutput=True)

    Tanh = mybir.ActivationFunctionType.Tanh

    with tile.TileContext(nc) as tc:
        with (
            tc.tile_pool(name="const", bufs=1) as cpool,
            tc.tile_pool(name="mm", bufs=1, space="PSUM") as mpool,
            tc.tile_pool(name="tailp", bufs=2, space="PSUM") as tpool,
            tc.tile_pool(name="sums", bufs=4) as spool,
            tc.tile_pool(name="outs", bufs=6) as opool,
            tc.tile_pool(name="tails", bufs=2) as tspool,
            tc.For_i(0, loop_k, 1) if loop_k else contextlib.nullcontext(),
        ):
            # one DMA per k-tile so matmul kk can start as soon as its
            # k-tile lands (pipelines the load under the matmul chain)
            inp_b = cpool.tile([128, KT * IC16], bf16, name="inp_b")
            for kk in range(KT):
                nc.sync.dma_start(
                    inp_b[:, kk * IC16 : (kk + 1) * IC16],
                    inp16_d[kk * 128 : (kk + 1) * 128, :],
                )
            aux_b = cpool.tile([128, KT * 2], f32, name="aux_b")
            nc.sync.dma_start(
                aux_b[:].rearrange("p (t c) -> p t c", t=KT),
                aux_d.rearrange("(t p) c -> p t c", p=128),
            )
            inda_b = cpool.tile([NTAIL, TAILN], bf16, name="inda_b")
            nc.sync.dma_start(inda_b[:], inda_d[:])
            indb_b = cpool.tile([128, TAILN], bf16, name="indb_b")
            nc.sync.dma_start(indb_b[:], indb_d[:])
            # block kk occupies cols [kk*IC16, (kk+1)*IC16)
            ht_t = [inp_b[:, kk * IC16 : kk * IC16 + S] for kk in range(KT)]
            fcb_t = [aux_b[:, c * 2 : c * 2 + 1] for c in range(OC)]

            for c in range(OC):
                pm1 = mpool.tile([128, S], f32, name="pm1")
                pm2 = mpool.tile([128, S], f32, name="pm2")
                for kk in range(KT):
                    nc.tensor.matmul(
                        pm1[:],
                        inp_b[
                            :, kk * IC16 + W1C + c * 128 : kk * IC16 + W1C + (c + 1) * 128
                        ],
                        ht_t[kk],
                        start=(kk == 0),
                        stop=(kk == KT - 1),
                    )
                for kk in range(KT):
                    nc.tensor.matmul(
                        pm2[:],
                        inp_b[
                            :, kk * IC16 + W2C + c * 128 : kk * IC16 + W2C + (c + 1) * 128
                        ],
                        ht_t[kk],
                        start=(kk == 0),
                        stop=(kk == KT - 1),
                    )
                # transposed tail rows: p1T[s, f] / p2T[s, f] for s = TI..S-1
                # (stationary = ht tail columns, moving = W block)
                p1t = mpool.tile([128, 128], f32, name="p1t")
                p2t = mpool.tile([128, 128], f32, name="p2t")
                for kk in range(KT):
                    nc.tensor.matmul(
                        p1t[:NTAIL, :],
                        inp_b[:, kk * IC16 + TI : kk * IC16 + S],
                        inp_b[
                            :, kk * IC16 + W1C + c * 128 : kk * IC16 + W1C + (c + 1) * 128
                        ],
                        start=(kk == 0),
                        stop=(kk == KT - 1),
                    )
                for kk in range(KT):
                    nc.tensor.matmul(
                        p2t[:NTAIL, :],
                        inp_b[:, kk * IC16 + TI : kk * IC16 + S],
                        inp_b[
                            :, kk * IC16 + W2C + c * 128 : kk * IC16 + W2C + (c + 1) * 128
                        ],
                        start=(kk == 0),
                        stop=(kk == KT - 1),
                    )

                # p1 stays f32: the tensor_scalar per-partition scalar operand
                # is exempt from the 16-bit requirement of DVE 2x mode.
                p1 = cpool.tile([128, S], f32, name=f"p1_{c}")
                # q2 = bf16(pm2 + fcb); col S is a pad column read (only) by
                # the even-length extension of odd-L segments.
                q2 = cpool.tile([128, S + 1], bf16, name=f"q2_{c}")
                # q2s[k] = q2[k+1]: odd-i segments read q2s at even offset i-1
                q2s = cpool.tile([128, S], bf16, name=f"q2s_{c}")
                nc.vector.tensor_copy(p1[:], pm1[:])
                nc.vector.tensor_scalar_add(q2[:, :S], pm2[:], fcb_t[c])
                nc.vector.memset(q2[:, S : S + 1], 0.0)
                nc.vector.tensor_copy(q2s[:], q2[:, 1 : S + 1])

                # tail stationary stacks (bf16): STK_A = p1T rows,
                # STK_B = p2T rows + fc_b row at partition 127
                stka = cpool.tile([NTAIL, 128], bf16, name=f"stka_{c}")
                stkb = cpool.tile([128, 128], bf16, name=f"stkb_{c}")
                nc.vector.tensor_copy(stka[:], p1t[:NTAIL, :])
                nc.vector.tensor_copy(stkb[:NTAIL, :], p2t[:NTAIL, :])
                nc.sync.dma_start(stkb[NTAIL : NTAIL + 1, :], brow_d[c : c + 1, :])

                # middle segments: DVE adds + chunked ACT tanh + DMA
                for coff, csz, ops in chunks:
                    ot = spool.tile([128, cmax], bf16, name="ot")
                    ot2 = opool.tile([128, cmax], bf16, name="ot2")
                    for i, sel, soff, lp, d0 in ops:
                        src = q2s if sel else q2
                        nc.vector.tensor_scalar_add(
                            ot[:, d0 : d0 + lp],
                            src[:, soff : soff + lp],
                            p1[:, i : i + 1],
                        )
                    nc.scalar.activation(ot2[:, :csz], ot[:, :csz], Tanh)
                    nc.sync.dma_start(
                        out_d[c, :, coff : coff + csz], ot2[:, :csz]
                    )

                # tail: gather-matmul into PSUM, tanh straight out of PSUM
                ot2t = tspool.tile([128, TAILN], bf16, name="ot2t")
                half = ntp // 2
                for t in range(ntp):
                    t0 = t * TPCH
                    tc_ = min(TPCH, TAILN - t0)
                    tp = tpool.tile([128, TPCH], f32, name="tp")
                    for s in range(0, tc_, 512):
                        sz = min(512, tc_ - s)
                        nc.tensor.matmul(
                            tp[:, s : s + sz],
                            stka[:],
                            inda_b[:, t0 + s : t0 + s + sz],
                            start=True,
                            stop=False,
                        )
                        nc.tensor.matmul(
                            tp[:, s : s + sz],
                            stkb[:],
                            indb_b[:, t0 + s : t0 + s + sz],
                            start=False,
                            stop=True,
                        )
                    nc.scalar.activation(
                        ot2t[:, t0 : t0 + tc_], tp[:, :tc_], Tanh
                    )
                    if t == half - 1:
                        nc.sync.dma_start(
                            out_d[c, :, midp : midp + half * TPCH],
                            ot2t[:, : half * TPCH],
                        )
                nc.sync.dma_start(
                    out_d[c, :, midp + half * TPCH : midp + TAILN],
                    ot2t[:, half * TPCH :],
                )
    nc.compile()
    return nc


def _get_nc():
    if "nc" not in _NC_CACHE:
        _NC_CACHE["nc"] = _build_nc()
    return _NC_CACHE["nc"]


def _make_in_maps(hidden_state, fc_w, fc_b):
    import ml_dtypes

    ii, jj = _tail_pairs()
    inda = np.zeros((NTAIL, TAILN), dtype=ml_dtypes.bfloat16)
    inda[ii - TI, np.arange(TAILN)] = 1
    indb = np.zeros((128, TAILN), dtype=ml_dtypes.bfloat16)
    indb[jj - TI, np.arange(TAILN)] = 1
    indb[NTAIL, :] = 1  # bias row

    in_maps = []
    for k in range(8):
        b, h0 = k // 2, 384 * (k % 2)
        inp16 = np.empty((H, IC16), dtype=ml_dtypes.bfloat16)
        inp16[:, :S] = hidden_state[b].T.astype(ml_dtypes.bfloat16)
        inp16[:, W1C : W1C + 384] = fc_w[h0 : h0 + 384, :H].T.astype(
            ml_dtypes.bfloat16
        )
        inp16[:, W2C : W2C + 384] = fc_w[h0 : h0 + 384, H:].T.astype(
            ml_dtypes.bfloat16
        )
        aux = np.zeros((H, 2), dtype=np.float32)
        aux[: 128 * OC, 0] = fc_b[h0 : h0 + 384]
        brow = (
            fc_b[h0 : h0 + 384].reshape(OC, 128).astype(ml_dtypes.bfloat16)
        )
        in_maps.append(
            dict(inp16=inp16, aux=aux, ind_a=inda, ind_b=indb, brow=brow)
        )
    return in_maps


def kernel(hidden_state, fc_w, fc_b, _trace=False, **_trace_kwargs):
    from concourse.bass_utils import run_bass_kernel_spmd

    hidden_state = np.asarray(hidden_state, dtype=np.float32)
    fc_w = np.asarray(fc_w, dtype=np.float32)
    fc_b = np.asarray(fc_b, dtype=np.float32)

    in_maps = _make_in_maps(hidden_state, fc_w, fc_b)
    nc = _get_nc()
    res = run_bass_kernel_spmd(
        nc, in_maps, core_ids=list(range(8)), trace=_trace, **_trace_kwargs
    )
    LAST["res"] = res

    ppad, _, _, idx = _layout()
    full = np.empty((B, H, P), dtype=np.float32)
    for k in range(8):
        b, h0 = k // 2, 384 * (k % 2)
        o = np.asarray(res.results[k]["out"]).reshape(384, ppad)
        full[b, h0 : h0 + 384] = o[:, idx].astype(np.float32)
    return np.ascontiguousarray(full.transpose(0, 2, 1))


# revision 10
# speedup vs baseline: 1.5456x; 1.5456x over previous
"""Trainium2 Bass kernel for ConcatHandshaking.

out[b, p, :] = tanh(hidden[b, i_p] @ W1.T + hidden[b, j_p] @ W2.T + fc_b)
for the S*(S+1)/2 upper-triangular pairs (i_p, j_p), i-major order.

Device layout: output features (H=768) on SBUF partitions, pair index on the
free dim.  Per stripe (128 features) the pre-activation for pair (i, j) is
q2[:, j] + p1[:, i]; the work is split by segment length:

- middle segments (i < TI): one DVE tensor_scalar_add per segment (runs in
  2x perf mode: bf16 tensor operands, f32 per-partition scalar), then one
  big ACT tanh per ~4K-column chunk, then ~1MB DMAs.  Segment lengths are
  padded to even so every bf16 AP is 4B-aligned (host discards pad cols);
  odd-start reads use q2s, a one-column-shifted copy of q2.
- tail segments (i >= TI = 129, lengths <= 127): per-op overhead would
  drown DVE (and ACT), so the tensor engine computes them instead:
  psum[f, t] = sum_k STK[k, f] * IND[k, t] where STK stacks [p1T rows;
  p2T rows; fc_b row] (built on-chip via transposed matmuls) and IND is a
  constant 0/1 bf16 matrix streamed from HBM (IND_A: i-indicators,
  IND_B: j-indicators + ones row for the bias).  ACT tanh reads the f32
  PSUM chunks directly and writes bf16 to SBUF.

Everything after PSUM is bf16: halves DMA write traffic (HBM ~358GB/s/core
would otherwise bound at ~140us) and doubles DVE throughput.

Sharding (8 cores): core k handles batch b = k//2 and output-feature rows
[384*(k%2), 384*(k%2)+384) -> 3 stripes of [128 features, P pairs] each.
Per-core DRAM output is (3, 128, PPAD) bf16; host gathers the packed
columns, upcasts to f32 and transposes.
"""

import sys

import numpy as np

for _p in ("/opt/trn_rl_repo",):
    if _p not in sys.path:
        sys.path.insert(0, _p)

B, S, H = 4, 256, 768
P = S * (S + 1) // 2  # 32896
KT = H // 128  # 6 k-tiles
OC = 3  # o-chunks (of 128) per core
# bf16 packed matmul input columns: [ ht (S) | w1t (384) | w2t (384) ]
W1C = S
W2C = S + 128 * OC
IC16 = S + 2 * 128 * OC  # 1024

TI = 129  # tail threshold: segments i >= TI go through the tensor engine
NTAIL = S - TI  # 127 tail i-rows (and 127 tail j-values TI..255)
TAILN = NTAIL * (NTAIL + 1) // 2  # 8128 packed tail columns per stripe
TPCH = 1024  # tail PSUM chunk columns (2 banks)
NCHUNK = 6  # middle tanh/DMA chunks per stripe (~1MB bf16 DMAs)

_NC_CACHE = {}
_LAYOUT_CACHE = {}
LAST = {}


def _layout(nchunk=NCHUNK, lead_split=True):
    """Padded middle layout + chunking + packed tail, and the host gather map.

    Middle segment i (i < TI) holds pairs (i, j) j=i..S-1, length L=S-i,
    padded to even Lp.  Chunk boundaries snap to segment starts.  The tail
    region (packed, TAILN cols) sits after the middle.

    Returns (PPAD, MIDP, chunks, idx): chunks is a list per middle chunk of
    (coff, csz, [(i, src_sel, src_off, Lp, dst0), ...]); idx maps packed
    column -> padded column for the host-side gather.
    """
    key = (nchunk, lead_split)
    if key in _LAYOUT_CACHE:
        return _LAYOUT_CACHE[key]
    segs = []
    pos = 0
    for i in range(TI):
        L = S - i
        Lp = L + (L & 1)
        segs.append((i, pos, L, Lp))
        pos += Lp
    midp = pos  # 24832
    ppad = midp + TAILN  # 32960
    targets = [round(midp * t / nchunk) for t in range(1, nchunk)]
    bounds = [0]
    for t in targets:
        b = min((s[1] for s in segs), key=lambda x: abs(x - t))
        if b > bounds[-1]:
            bounds.append(b)
    bounds.append(midp)
    if lead_split:
        # split the first chunk in half for earlier pipeline start
        half = min(
            (s[1] for s in segs if 0 < s[1] < bounds[1]),
            key=lambda x: abs(x - bounds[1] // 2),
        )
        bounds = [0, half] + bounds[1:]
    chunks = []
    idx = np.empty(P, dtype=np.int64)
    poff = 0
    si = 0
    for c0, c1 in zip(bounds[:-1], bounds[1:]):
        ops = []
        while si < len(segs) and segs[si][1] < c1:
            i, dstart, L, Lp = segs[si]
            idx[poff : poff + L] = dstart + np.arange(L)
            poff += L
            if i % 2 == 0:
                ops.append((i, 0, i, Lp, dstart - c0))
            else:
                ops.append((i, 1, i - 1, Lp, dstart - c0))
            si += 1
        chunks.append((c0, c1 - c0, ops))
    idx[poff:] = midp + np.arange(TAILN)
    _LAYOUT_CACHE[key] = (ppad, midp, chunks, idx)
    return _LAYOUT_CACHE[key]


def _tail_pairs():
    ii, jj = [], []
    for i in range(TI, S):
        for j in range(i, S):
            ii.append(i)
            jj.append(j)
    return np.array(ii), np.array(jj)


def _build_nc(loop_k=None, nchunk=NCHUNK):
    import contextlib

    import concourse.bacc as bacc
    import concourse.bass as bass
    import concourse.mybir as mybir
    import concourse.tile as tile

    f32 = mybir.dt.float32
    bf16 = mybir.dt.bfloat16
    ppad, midp, chunks, _ = _layout(nchunk)
    cmax = max(c[1] for c in chunks)
    ntp = (TAILN + TPCH - 1) // TPCH  # tail psum chunks per stripe

    # Bacc (not raw Bass): its compile() runs generate_event_semaphores,
    # which splits multi-sem waits to satisfy TRN2's 1-wait-per-instruction.
    nc = bacc.Bacc()

    inp16_d = nc.declare_dram_parameter("inp16", [H, IC16], bf16, isOutput=False)
    # f32 side data: col 0 = fcb (rows 0:384), col 1 = zeros
    aux_d = nc.declare_dram_parameter("aux", [H, 2], f32, isOutput=False)
    # constant 0/1 indicator matrices for the tail gather-matmul
    # 128 partitions (last row zero): odd partition counts degenerate into a
    # single-SDMA-engine DMA (measured: a [127, N] load put all rows on E64)
    inda_d = nc.declare_dram_parameter("ind_a", [128, TAILN], bf16, isOutput=False)
    indb_d = nc.declare_dram_parameter("ind_b", [128, TAILN], bf16, isOutput=False)
    # per-stripe fc_b as a row (partition 127 of STK_B)
    brow_d = nc.declare_dram_parameter("brow", [OC, 128], bf16, isOutput=False)
    out_d = nc.declare_dram_parameter("out", [OC, 128, ppad], bf16, isOutput=True)

    Tanh = mybir.ActivationFunctionType.Tanh

    with tile.TileContext(nc) as tc:
        with (
            tc.tile_pool(name="const", bufs=1) as cpool,
            tc.tile_pool(name="mm", bufs=1, space="PSUM") as mpool,
            tc.tile_pool(name="tailp", bufs=2, space="PSUM") as tpool,
            tc.tile_pool(name="sums", bufs=4) as spool,
            tc.tile_pool(name="outs", bufs=6) as opool,
            tc.tile_pool(name="tails", bufs=2) as tspool,
            tc.For_i(0, loop_k, 1) if loop_k else contextlib.nullcontext(),
        ):
            # one DMA per k-tile so matmul kk can start as soon as its
            # k-tile lands (pipelines the load under the matmul chain)
            inp_b = cpool.tile([128, KT * IC16], bf16, name="inp_b")
            for kk in range(KT):
                nc.sync.dma_start(
                    inp_b[:, kk * IC16 : (kk + 1) * IC16],
                    inp16_d[kk * 128 : (kk + 1) * 128, :],
                )
            aux_b = cpool.tile([128, KT * 2], f32, name="aux_b")
            nc.sync.dma_start(
                aux_b[:].rearrange("p (t c) -> p t c", t=KT),
                aux_d.rearrange("(t p) c -> p t c", p=128),
            )
            inda_b = cpool.tile([128, TAILN], bf16, name="inda_b")
            nc.sync.dma_start(inda_b[:], inda_d[:])
            indb_b = cpool.tile([128, TAILN], bf16, name="indb_b")
            nc.sync.dma_start(indb_b[:], indb_d[:])
            # tiny early tanh: pulls the ~2.7us ACT table load off the
            # critical path (runs during input load + matmuls)
            warm = cpool.tile([128, 2], bf16, name="warm")
            nc.scalar.activation(
                warm[:], aux_b[:, :2], mybir.ActivationFunctionType.Tanh
            )
            # block kk occupies cols [kk*IC16, (kk+1)*IC16)
            ht_t = [inp_b[:, kk * IC16 : kk * IC16 + S] for kk in range(KT)]
            fcb_t = [aux_b[:, c * 2 : c * 2 + 1] for c in range(OC)]

            for c in range(OC):
                pm1 = mpool.tile([128, S], f32, name="pm1")
                pm2 = mpool.tile([128, S], f32, name="pm2")
                for kk in range(KT):
                    nc.tensor.matmul(
                        pm1[:],
                        inp_b[
                            :, kk * IC16 + W1C + c * 128 : kk * IC16 + W1C + (c + 1) * 128
                        ],
                        ht_t[kk],
                        start=(kk == 0),
                        stop=(kk == KT - 1),
                    )
                for kk in range(KT):
                    nc.tensor.matmul(
                        pm2[:],
                        inp_b[
                            :, kk * IC16 + W2C + c * 128 : kk * IC16 + W2C + (c + 1) * 128
                        ],
                        ht_t[kk],
                        start=(kk == 0),
                        stop=(kk == KT - 1),
                    )
                # transposed tail rows: p1T[s, f] / p2T[s, f] for s = TI..S-1
                # (stationary = ht tail columns, moving = W block)
                p1t = mpool.tile([128, 128], f32, name="p1t")
                p2t = mpool.tile([128, 128], f32, name="p2t")
                for kk in range(KT):
                    nc.tensor.matmul(
                        p1t[:NTAIL, :],
                        inp_b[:, kk * IC16 + TI : kk * IC16 + S],
                        inp_b[
                            :, kk * IC16 + W1C + c * 128 : kk * IC16 + W1C + (c + 1) * 128
                        ],
                        start=(kk == 0),
                        stop=(kk == KT - 1),
                    )
                for kk in range(KT):
                    nc.tensor.matmul(
                        p2t[:NTAIL, :],
                        inp_b[:, kk * IC16 + TI : kk * IC16 + S],
                        inp_b[
                            :, kk * IC16 + W2C + c * 128 : kk * IC16 + W2C + (c + 1) * 128
                        ],
                        start=(kk == 0),
                        stop=(kk == KT - 1),
                    )

                # p1 stays f32: the tensor_scalar per-partition scalar operand
                # is exempt from the 16-bit requirement of DVE 2x mode.
                p1 = cpool.tile([128, S], f32, name=f"p1_{c}")
                # q2 = bf16(pm2 + fcb); col S is a pad column read (only) by
                # the even-length extension of odd-L segments.
                q2 = cpool.tile([128, S + 1], bf16, name=f"q2_{c}")
                # q2s[k] = q2[k+1]: odd-i segments read q2s at even offset i-1
                q2s = cpool.tile([128, S], bf16, name=f"q2s_{c}")
                nc.vector.tensor_copy(p1[:], pm1[:])
                nc.vector.tensor_scalar_add(q2[:, :S], pm2[:], fcb_t[c])
                nc.vector.memset(q2[:, S : S + 1], 0.0)
                nc.vector.tensor_copy(q2s[:], q2[:, 1 : S + 1])

                # tail stationary stacks (bf16): STK_A = p1T rows,
                # STK_B = p2T rows + fc_b row at partition 127
                stka = cpool.tile([128, 128], bf16, name=f"stka_{c}")
                stkb = cpool.tile([128, 128], bf16, name=f"stkb_{c}")
                # compute-engine APs must start at a 32-aligned partition:
                # zero rows 96..127 first, the copy overwrites 96..126
                nc.vector.memset(stka[96:128, :], 0.0)
                nc.vector.tensor_copy(stka[:NTAIL, :], p1t[:NTAIL, :])
                nc.vector.tensor_copy(stkb[:NTAIL, :], p2t[:NTAIL, :])
                nc.sync.dma_start(stkb[NTAIL : NTAIL + 1, :], brow_d[c : c + 1, :])

                # middle segments: DVE adds + chunked ACT tanh + DMA
                for coff, csz, ops in chunks:
                    ot = spool.tile([128, cmax], bf16, name="ot")
                    ot2 = opool.tile([128, cmax], bf16, name="ot2")
                    for i, sel, soff, lp, d0 in ops:
                        src = q2s if sel else q2
                        nc.vector.tensor_scalar_add(
                            ot[:, d0 : d0 + lp],
                            src[:, soff : soff + lp],
                            p1[:, i : i + 1],
                        )
                    nc.scalar.activation(ot2[:, :csz], ot[:, :csz], Tanh)
                    nc.sync.dma_start(
                        out_d[c, :, coff : coff + csz], ot2[:, :csz]
                    )

                # tail: gather-matmul into PSUM, tanh straight out of PSUM;
                # output DMA every 2 psum chunks to keep the drain short
                ot2t = tspool.tile([128, TAILN], bf16, name="ot2t")
                for t in range(ntp):
                    t0 = t * TPCH
                    tc_ = min(TPCH, TAILN - t0)
                    tp = tpool.tile([128, TPCH], f32, name="tp")
                    for s in range(0, tc_, 512):
                        sz = min(512, tc_ - s)
                        nc.tensor.matmul(
                            tp[:, s : s + sz],
                            stka[:],
                            inda_b[:, t0 + s : t0 + s + sz],
                            start=True,
                            stop=False,
                        )
                        nc.tensor.matmul(
                            tp[:, s : s + sz],
                            stkb[:],
                            indb_b[:, t0 + s : t0 + s + sz],
                            start=False,
                            stop=True,
                        )
                    nc.scalar.activation(
                        ot2t[:, t0 : t0 + tc_], tp[:, :tc_], Tanh
                    )
                    if t % 2 == 1 or t == ntp - 1:
                        d0 = (t // 2) * 2 * TPCH
                        d1 = t0 + tc_
                        nc.sync.dma_start(
                            out_d[c, :, midp + d0 : midp + d1],
                            ot2t[:, d0:d1],
                        )
    nc.compile()
    return nc


def _get_nc():
    if "nc" not in _NC_CACHE:
        _NC_CACHE["nc"] = _build_nc()
    return _NC_CACHE["nc"]


def _make_in_maps(hidden_state, fc_w, fc_b):
    import ml_dtypes

    ii, jj = _tail_pairs()
    inda = np.zeros((128, TAILN), dtype=ml_dtypes.bfloat16)
    inda[ii - TI, np.arange(TAILN)] = 1
    indb = np.zeros((128, TAILN), dtype=ml_dtypes.bfloat16)
    indb[jj - TI, np.arange(TAILN)] = 1
    indb[NTAIL, :] = 1  # bias row

    in_maps = []
    for k in range(8):
        b, h0 = k // 2, 384 * (k % 2)
        inp16 = np.empty((H, IC16), dtype=ml_dtypes.bfloat16)
        inp16[:, :S] = hidden_state[b].T.astype(ml_dtypes.bfloat16)
        inp16[:, W1C : W1C + 384] = fc_w[h0 : h0 + 384, :H].T.astype(
            ml_dtypes.bfloat16
        )
        inp16[:, W2C : W2C + 384] = fc_w[h0 : h0 + 384, H:].T.astype(
            ml_dtypes.bfloat16
        )
        aux = np.zeros((H, 2), dtype=np.float32)
        aux[: 128 * OC, 0] = fc_b[h0 : h0 + 384]
        brow = (
            fc_b[h0 : h0 + 384].reshape(OC, 128).astype(ml_dtypes.bfloat16)
        )
        in_maps.append(
            dict(inp16=inp16, aux=aux, ind_a=inda, ind_b=indb, brow=brow)
        )
    return in_maps


def kernel(hidden_state, fc_w, fc_b, _trace=False, **_trace_kwargs):
    from concourse.bass_utils import run_bass_kernel_spmd

    hidden_state = np.asarray(hidden_state, dtype=np.float32)
    fc_w = np.asarray(fc_w, dtype=np.float32)
    fc_b = np.asarray(fc_b, dtype=np.float32)

    in_maps = _make_in_maps(hidden_state, fc_w, fc_b)
    nc = _get_nc()
    res = run_bass_kernel_spmd(
        nc, in_maps, core_ids=list(range(8)), trace=_trace, **_trace_kwargs
    )
    LAST["res"] = res

    ppad, _, _, idx = _layout()
    full = np.empty((B, H, P), dtype=np.float32)
    for k in range(8):
        b, h0 = k // 2, 384 * (k % 2)
        o = np.asarray(res.results[k]["out"]).reshape(384, ppad)
        full[b, h0 : h0 + 384] = o[:, idx].astype(np.float32)
    return np.ascontiguousarray(full.transpose(0, 2, 1))
